# revision 3
# baseline (speedup 1.0000x reference)
"""Trainium2 Bass kernel for nn_GCK3x3Layer: 3x3 VALID conv, 256->256 ch, 258x258.

result = kernelsL @ im2col_3x3(input); input (1,256,258,258) f32,
kernelsL (256, 2304) f32 -> output (1, 256, 256, 256) f32.

Strategy: spatial-parallel across 8 NeuronCores. Each core gets a 34-row
input slab (32 output rows + 2 halo rows) and the full weight matrix, and
computes all 256 output channels for its strip via implicit-GEMM:
for each of 9 filter taps and 2 input-channel blocks, a [128,128]x[128,512]
matmul accumulating into PSUM (K = 2304 contraction in 18 chunks of 128,
N = 512 = two output rows of 256 pixels).
"""

import os
import sys
from contextlib import ExitStack

import numpy as np

for _p in (
    "/root/.axon_site",
    "/root/.axon_site/_ro/trn_rl_repo",
    "/root/.axon_site/_ro/pypackages",
    "/opt/trn_rl_repo",
):
    if os.path.isdir(_p) and _p not in sys.path:
        sys.path.append(_p)

import concourse.bass as bass  # noqa: E402
import concourse.tile as tile  # noqa: E402
from concourse import bacc, mybir  # noqa: E402
from concourse.bass_utils import run_bass_kernel_spmd  # noqa: E402

IN_C = 256
OUT_C = 256
H = 258
W = 258
H_OUT = H - 2  # 256
W_OUT = W - 2  # 256
NCORES = 8
ROWS_PER_CORE = H_OUT // NCORES  # 32
IN_ROWS = ROWS_PER_CORE + 2  # 34
P = 128
ICB = IN_C // P  # 2 input-channel blocks
OCB = OUT_C // P  # 2 output-channel blocks
KB = ICB * 9  # 18 contraction blocks of 128
PAIRS = ROWS_PER_CORE // 2  # 16 output-row pairs (N=512 per matmul)

F32 = mybir.dt.float32


def build(mm_dtype=mybir.dt.float32r, repeat=1, x_chunk_rows=6):
    """Build + compile the per-core Bass program (identical on all cores)."""
    nc = bacc.Bacc(
        "TRN2", target_bir_lowering=False, debug=False, num_devices=NCORES
    )
    x = nc.dram_tensor("x", [IN_C, IN_ROWS * W], F32, kind="ExternalInput")
    wT = nc.dram_tensor("wT", [9 * IN_C, OUT_C], F32, kind="ExternalInput")
    y = nc.dram_tensor(
        "y", [OUT_C, ROWS_PER_CORE * W_OUT], F32, kind="ExternalOutput"
    )

    xv = x.rearrange("(b p) (r c) -> p b r c", p=P, c=W)
    wv = wT.rearrange("(b p) m -> p b m", p=P)
    if mm_dtype == mybir.dt.float32r:
        # f32r is bit-compatible with f32; declaring the SBUF tiles f32r
        # (and bitcasting the DMA source) satisfies the walrus requirement
        # that FP32r matmul operands come from an f32r-typed producer.
        xv = xv.bitcast(mm_dtype)
        wv = wv.bitcast(mm_dtype)

    with tile.TileContext(nc) as tc:
        with ExitStack() as ctx:
            xpool = ctx.enter_context(tc.tile_pool(name="xp", bufs=1))
            wpool = ctx.enter_context(tc.tile_pool(name="wp", bufs=1))
            pspool = ctx.enter_context(
                tc.tile_pool(name="ps", bufs=8, space="PSUM")
            )
            opool = ctx.enter_context(tc.tile_pool(name="op", bufs=4))

            w_sb = wpool.tile([P, KB, OUT_C], mm_dtype)
            nc.sync.dma_start(w_sb[:], wv)

            x_sb = xpool.tile([P, ICB, IN_ROWS, W], mm_dtype)
            r0 = 0
            while r0 < IN_ROWS:
                r1 = min(r0 + x_chunk_rows, IN_ROWS)
                for b in range(ICB):
                    nc.sync.dma_start(x_sb[:, b, r0:r1, :], xv[:, b, r0:r1, :])
                r0 = r1

            for _rep in range(repeat):
                for pr in range(PAIRS):
                    for ocb in range(OCB):
                        ps = pspool.tile([P, 2, W_OUT], F32)
                        ki = 0
                        for icb in range(ICB):
                            for pos in range(9):
                                dy, dx = divmod(pos, 3)
                                lhsT = w_sb[
                                    :, pos * ICB + icb, ocb * P : (ocb + 1) * P
                                ]
                                rhs = x_sb[
                                    :,
                                    icb,
                                    2 * pr + dy : 2 * pr + dy + 2,
                                    dx : dx + W_OUT,
                                ]
                                nc.tensor.matmul(
                                    ps[:, :, :],
                                    lhsT,
                                    rhs,
                                    start=(ki == 0),
                                    stop=(ki == KB - 1),
                                )
                                ki += 1
                        ot = opool.tile([P, 2 * W_OUT], F32)
                        nc.vector.tensor_copy(ot[:], ps.rearrange("p a b -> p (a b)"))
                        nc.sync.dma_start(
                            y[
                                ocb * P : (ocb + 1) * P,
                                pr * 2 * W_OUT : (pr + 1) * 2 * W_OUT,
                            ],
                            ot[:],
                        )
    nc.compile()
    return nc


_NC_CACHE = {}


def _get_nc():
    if "nc" not in _NC_CACHE:
        _NC_CACHE["nc"] = build()
    return _NC_CACHE["nc"]


def make_in_maps(input, kernelsL):
    inp = np.asarray(input, dtype=np.float32).reshape(IN_C, H, W)
    w = np.asarray(kernelsL, dtype=np.float32)
    # wT[pos*256 + ic, oc] = kernelsL[oc, ic*9 + pos]
    wT = np.ascontiguousarray(
        w.reshape(OUT_C, IN_C, 9).transpose(2, 1, 0).reshape(9 * IN_C, OUT_C)
    )
    in_maps = []
    for c in range(NCORES):
        r0 = c * ROWS_PER_CORE
        strip = np.ascontiguousarray(inp[:, r0 : r0 + IN_ROWS, :]).reshape(
            IN_C, IN_ROWS * W
        )
        in_maps.append({"x": strip, "wT": wT})
    return in_maps


def assemble(results):
    out = np.empty((OUT_C, H_OUT, W_OUT), dtype=np.float32)
    for c in range(NCORES):
        out[:, c * ROWS_PER_CORE : (c + 1) * ROWS_PER_CORE, :] = results[c][
            "y"
        ].reshape(OUT_C, ROWS_PER_CORE, W_OUT)
    return out.reshape(1, OUT_C, H_OUT, W_OUT)


def kernel(input, kernelsL):
    in_maps = make_in_maps(input, kernelsL)
    nc = _get_nc()
    res = run_bass_kernel_spmd(nc, in_maps, core_ids=list(range(NCORES)))
    return assemble(res.results)


# revision 7
# speedup vs baseline: 186.6065x; 186.6065x over previous
"""Trainium2 Bass kernel for nn_GCK3x3Layer: 3x3 VALID conv, 256->256 ch, 258x258.

result = kernelsL @ im2col_3x3(input); input (1,256,258,258) f32,
kernelsL (256, 2304) f32 -> output (1, 256, 256, 256) f32.

Strategy: spatial-parallel across 8 NeuronCores. Each core gets a 34-row
input slab (32 output rows + 2 halo rows) and the full weight matrix, and
computes all 256 output channels for its strip via implicit-GEMM:
for each of 9 filter taps and 2 input-channel blocks, a [128,128]x[128,512]
matmul accumulating into PSUM (K = 2304 contraction in 18 chunks of 128,
N = 512 = two output rows of 256 pixels).
"""

import os
import sys
from contextlib import ExitStack

import numpy as np

for _p in (
    "/root/.axon_site",
    "/root/.axon_site/_ro/trn_rl_repo",
    "/root/.axon_site/_ro/pypackages",
    "/opt/trn_rl_repo",
):
    if os.path.isdir(_p) and _p not in sys.path:
        sys.path.append(_p)

import concourse.bass as bass  # noqa: E402
import concourse.tile as tile  # noqa: E402
from concourse import bacc, mybir  # noqa: E402
from concourse.bass_utils import run_bass_kernel_spmd  # noqa: E402

IN_C = 256
OUT_C = 256
H = 258
W = 258
H_OUT = H - 2  # 256
W_OUT = W - 2  # 256
NCORES = 8
ROWS_PER_CORE = H_OUT // NCORES  # 32
IN_ROWS = ROWS_PER_CORE + 2  # 34
P = 128
ICB = IN_C // P  # 2 input-channel blocks
OCB = OUT_C // P  # 2 output-channel blocks
KB = ICB * 9  # 18 contraction blocks of 128
PAIRS = ROWS_PER_CORE // 2  # 16 output-row pairs (N=512 per matmul)

F32 = mybir.dt.float32


def build(mm_dtype=mybir.dt.float32r, repeat=1, x_chunk_rows=6, loop_repeat=1):
    """Build + compile the per-core Bass program (identical on all cores).

    repeat: python-unrolled repetitions of the compute pass (dev timing).
    loop_repeat: hardware For_i repetitions of the whole pass (dev timing).
    """
    nc = bacc.Bacc(
        "TRN2", target_bir_lowering=False, debug=False, num_devices=NCORES
    )
    x = nc.dram_tensor("x", [IN_C, IN_ROWS * W], F32, kind="ExternalInput")
    wT = nc.dram_tensor("wT", [9 * IN_C, OUT_C], F32, kind="ExternalInput")
    y = nc.dram_tensor(
        "y", [OUT_C, ROWS_PER_CORE * W_OUT], F32, kind="ExternalOutput"
    )

    xv = x.rearrange("(b p) (r c) -> p b r c", p=P, c=W)
    wv = wT.rearrange("(b p) m -> p b m", p=P)
    if mm_dtype == mybir.dt.float32r:
        # f32r is bit-compatible with f32; declaring the SBUF tiles f32r
        # (and bitcasting the DMA source) satisfies the walrus requirement
        # that FP32r matmul operands come from an f32r-typed producer.
        xv = xv.bitcast(mm_dtype)
        wv = wv.bitcast(mm_dtype)

    with tile.TileContext(nc) as tc:
        with ExitStack() as ctx:
            xpool = ctx.enter_context(
                tc.tile_pool(name="xp", bufs=2 if loop_repeat > 1 else 1)
            )
            wpool = ctx.enter_context(tc.tile_pool(name="wp", bufs=1))
            pspool = ctx.enter_context(
                tc.tile_pool(name="ps", bufs=8, space="PSUM")
            )
            opool = ctx.enter_context(tc.tile_pool(name="op", bufs=4))

            w_sb = wpool.tile([P, KB, OUT_C], mm_dtype)
            nc.sync.dma_start(w_sb[:], wv)

            def _one_pass():
                x_sb = xpool.tile([P, ICB, IN_ROWS, W], mm_dtype, name="x_sb")
                r0 = 0
                while r0 < IN_ROWS:
                    r1 = min(r0 + x_chunk_rows, IN_ROWS)
                    for b in range(ICB):
                        nc.sync.dma_start(
                            x_sb[:, b, r0:r1, :], xv[:, b, r0:r1, :]
                        )
                    r0 = r1
                for pr in range(PAIRS):
                    for ocb in range(OCB):
                        ps = pspool.tile([P, 2, W_OUT], F32)
                        ki = 0
                        for icb in range(ICB):
                            for pos in range(9):
                                dy, dx = divmod(pos, 3)
                                lhsT = w_sb[
                                    :, pos * ICB + icb, ocb * P : (ocb + 1) * P
                                ]
                                rhs = x_sb[
                                    :,
                                    icb,
                                    2 * pr + dy : 2 * pr + dy + 2,
                                    dx : dx + W_OUT,
                                ]
                                nc.tensor.matmul(
                                    ps[:, :, :],
                                    lhsT,
                                    rhs,
                                    start=(ki == 0),
                                    stop=(ki == KB - 1),
                                )
                                ki += 1
                        ot = opool.tile([P, 2 * W_OUT], F32)
                        nc.vector.tensor_copy(ot[:], ps.rearrange("p a b -> p (a b)"))
                        nc.sync.dma_start(
                            y[
                                ocb * P : (ocb + 1) * P,
                                pr * 2 * W_OUT : (pr + 1) * 2 * W_OUT,
                            ],
                            ot[:],
                        )

            if loop_repeat > 1:
                with tc.For_i(0, loop_repeat, 1):
                    for _rep in range(repeat):
                        _one_pass()
            else:
                for _rep in range(repeat):
                    _one_pass()
    nc.compile()
    return nc


_NC_CACHE = {}


def _get_nc():
    if "nc" not in _NC_CACHE:
        _NC_CACHE["nc"] = build()
    return _NC_CACHE["nc"]


def make_in_maps(input, kernelsL):
    inp = np.asarray(input, dtype=np.float32).reshape(IN_C, H, W)
    w = np.asarray(kernelsL, dtype=np.float32)
    # wT[pos*256 + ic, oc] = kernelsL[oc, ic*9 + pos]
    wT = np.ascontiguousarray(
        w.reshape(OUT_C, IN_C, 9).transpose(2, 1, 0).reshape(9 * IN_C, OUT_C)
    )
    in_maps = []
    for c in range(NCORES):
        r0 = c * ROWS_PER_CORE
        strip = np.ascontiguousarray(inp[:, r0 : r0 + IN_ROWS, :]).reshape(
            IN_C, IN_ROWS * W
        )
        in_maps.append({"x": strip, "wT": wT})
    return in_maps


def assemble(results):
    out = np.empty((OUT_C, H_OUT, W_OUT), dtype=np.float32)
    for c in range(NCORES):
        out[:, c * ROWS_PER_CORE : (c + 1) * ROWS_PER_CORE, :] = results[c][
            "y"
        ].reshape(OUT_C, ROWS_PER_CORE, W_OUT)
    return out.reshape(1, OUT_C, H_OUT, W_OUT)


def kernel(input, kernelsL):
    in_maps = make_in_maps(input, kernelsL)
    nc = _get_nc()
    res = run_bass_kernel_spmd(nc, in_maps, core_ids=list(range(NCORES)))
    return assemble(res.results)


# revision 9
# speedup vs baseline: 252.5294x; 1.3533x over previous
"""Trainium2 Bass kernel for nn_GCK3x3Layer: 3x3 VALID conv, 256->256 ch, 258x258.

result = kernelsL @ im2col_3x3(input); input (1,256,258,258) f32,
kernelsL (256, 2304) f32 -> output (1, 256, 256, 256) f32.

Strategy: spatial-parallel across 8 NeuronCores. Each core gets a 34-row
input slab (32 output rows + 2 halo rows) and the full weight matrix, and
computes all 256 output channels for its strip via implicit-GEMM:
for each of 9 filter taps and 2 input-channel blocks, a [128,128]x[128,512]
matmul accumulating into PSUM (K = 2304 contraction in 18 chunks of 128,
N = 512 = two output rows of 256 pixels).
"""

import os
import sys
from contextlib import ExitStack

import numpy as np

for _p in (
    "/root/.axon_site",
    "/root/.axon_site/_ro/trn_rl_repo",
    "/root/.axon_site/_ro/pypackages",
    "/opt/trn_rl_repo",
):
    if os.path.isdir(_p) and _p not in sys.path:
        sys.path.append(_p)

import concourse.bass as bass  # noqa: E402
import concourse.tile as tile  # noqa: E402
from concourse import bacc, mybir  # noqa: E402
from concourse.bass_utils import run_bass_kernel_spmd  # noqa: E402

IN_C = 256
OUT_C = 256
H = 258
W = 258
H_OUT = H - 2  # 256
W_OUT = W - 2  # 256
NCORES = 8
ROWS_PER_CORE = H_OUT // NCORES  # 32
IN_ROWS = ROWS_PER_CORE + 2  # 34
P = 128
ICB = IN_C // P  # 2 input-channel blocks
OCB = OUT_C // P  # 2 output-channel blocks
KB = ICB * 9  # 18 contraction blocks of 128
PAIRS = ROWS_PER_CORE // 2  # 16 output-row pairs (N=512 per matmul)

F32 = mybir.dt.float32


def build(mm_dtype=mybir.dt.float32r, repeat=1, x_chunk_rows=6, loop_repeat=1):
    """Build + compile the per-core Bass program (identical on all cores).

    repeat: python-unrolled repetitions of the compute pass (dev timing).
    loop_repeat: hardware For_i repetitions of the whole pass (dev timing).
    """
    nc = bacc.Bacc(
        "TRN2", target_bir_lowering=False, debug=False, num_devices=NCORES
    )
    x = nc.dram_tensor("x", [IN_C, IN_ROWS * W], F32, kind="ExternalInput")
    wT = nc.dram_tensor("wT", [9 * IN_C, OUT_C], F32, kind="ExternalInput")
    y = nc.dram_tensor(
        "y", [OUT_C, ROWS_PER_CORE * W_OUT], F32, kind="ExternalOutput"
    )

    xv = x.rearrange("(b p) (r c) -> p b r c", p=P, c=W)
    wv = wT.rearrange("(b p) m -> p b m", p=P)
    if mm_dtype == mybir.dt.float32r:
        # f32r is bit-compatible with f32; declaring the SBUF tiles f32r
        # (and bitcasting the DMA source) satisfies the walrus requirement
        # that FP32r matmul operands come from an f32r-typed producer.
        xv = xv.bitcast(mm_dtype)
        wv = wv.bitcast(mm_dtype)

    with tile.TileContext(nc) as tc:
        with ExitStack() as ctx:
            xpool = ctx.enter_context(
                tc.tile_pool(name="xp", bufs=2 if loop_repeat > 1 else 1)
            )
            wpool = ctx.enter_context(tc.tile_pool(name="wp", bufs=1))
            pspool = ctx.enter_context(
                tc.tile_pool(name="ps", bufs=8, space="PSUM")
            )
            opool = ctx.enter_context(tc.tile_pool(name="op", bufs=4))

            # HAM warmup: the PE clock is gated to 1.2 GHz until ~3.4us of
            # sustained activity. Fill the initial DMA wait (weights + first
            # input chunk) with throwaway fp32 matmuls on a zeroed tile so
            # the real f32r stream starts at the full 2.4 GHz. fp32 avoids
            # the f32r rounded-producer requirement; results are never read.
            warm = wpool.tile([P, P], F32, name="warm")
            nc.gpsimd.memset(warm[:], 0.0)
            wps = pspool.tile([P, 2, W_OUT], F32, name="ps", tag="ps")
            for _ in range(12):
                nc.tensor.matmul(
                    wps[:, 0, 0:P],
                    warm[:],
                    warm[:],
                    start=True,
                    stop=True,
                    skip_group_check=True,
                )

            # Split the weight load by out-channel half: the first
            # accumulation group only consumes ocb=0 columns, so compute can
            # start once the first half (~1.2MB) lands instead of waiting for
            # the full 2.3MB transfer; the ocb=1 half streams in behind it.
            w_sb = wpool.tile([P, KB, OUT_C], mm_dtype)
            nc.sync.dma_start(w_sb[:, :, 0:P], wv[:, :, 0:P])
            nc.sync.dma_start(w_sb[:, :, P:OUT_C], wv[:, :, P:OUT_C])

            def _one_pass():
                x_sb = xpool.tile([P, ICB, IN_ROWS, W], mm_dtype, name="x_sb")
                r0 = 0
                while r0 < IN_ROWS:
                    r1 = min(r0 + x_chunk_rows, IN_ROWS)
                    for b in range(ICB):
                        nc.sync.dma_start(
                            x_sb[:, b, r0:r1, :], xv[:, b, r0:r1, :]
                        )
                    r0 = r1
                for pr in range(PAIRS):
                    for ocb in range(OCB):
                        ps = pspool.tile([P, 2, W_OUT], F32)
                        ki = 0
                        for icb in range(ICB):
                            for pos in range(9):
                                dy, dx = divmod(pos, 3)
                                lhsT = w_sb[
                                    :, pos * ICB + icb, ocb * P : (ocb + 1) * P
                                ]
                                rhs = x_sb[
                                    :,
                                    icb,
                                    2 * pr + dy : 2 * pr + dy + 2,
                                    dx : dx + W_OUT,
                                ]
                                nc.tensor.matmul(
                                    ps[:, :, :],
                                    lhsT,
                                    rhs,
                                    start=(ki == 0),
                                    stop=(ki == KB - 1),
                                )
                                ki += 1
                        ot = opool.tile([P, 2 * W_OUT], F32)
                        nc.vector.tensor_copy(ot[:], ps.rearrange("p a b -> p (a b)"))
                        nc.sync.dma_start(
                            y[
                                ocb * P : (ocb + 1) * P,
                                pr * 2 * W_OUT : (pr + 1) * 2 * W_OUT,
                            ],
                            ot[:],
                        )

            if loop_repeat > 1:
                with tc.For_i(0, loop_repeat, 1):
                    for _rep in range(repeat):
                        _one_pass()
            else:
                for _rep in range(repeat):
                    _one_pass()
    nc.compile()
    return nc


_NC_CACHE = {}


def _get_nc():
    if "nc" not in _NC_CACHE:
        _NC_CACHE["nc"] = build()
    return _NC_CACHE["nc"]


def make_in_maps(input, kernelsL):
    inp = np.asarray(input, dtype=np.float32).reshape(IN_C, H, W)
    w = np.asarray(kernelsL, dtype=np.float32)
    # wT[pos*256 + ic, oc] = kernelsL[oc, ic*9 + pos]
    wT = np.ascontiguousarray(
        w.reshape(OUT_C, IN_C, 9).transpose(2, 1, 0).reshape(9 * IN_C, OUT_C)
    )
    in_maps = []
    for c in range(NCORES):
        r0 = c * ROWS_PER_CORE
        strip = np.ascontiguousarray(inp[:, r0 : r0 + IN_ROWS, :]).reshape(
            IN_C, IN_ROWS * W
        )
        in_maps.append({"x": strip, "wT": wT})
    return in_maps


def assemble(results):
    out = np.empty((OUT_C, H_OUT, W_OUT), dtype=np.float32)
    for c in range(NCORES):
        out[:, c * ROWS_PER_CORE : (c + 1) * ROWS_PER_CORE, :] = results[c][
            "y"
        ].reshape(OUT_C, ROWS_PER_CORE, W_OUT)
    return out.reshape(1, OUT_C, H_OUT, W_OUT)


def kernel(input, kernelsL):
    in_maps = make_in_maps(input, kernelsL)
    nc = _get_nc()
    res = run_bass_kernel_spmd(nc, in_maps, core_ids=list(range(NCORES)))
    return assemble(res.results)
